# revision 1
# baseline (speedup 1.0000x reference)
"""Trainium2 Bass kernel for a pre-LN transformer block (B=256, T=200, E=384).

Data-parallel over batch: 8 NeuronCores x 32 batches. Each core runs the full
block (LN1 -> QKV -> causal attention -> proj+residual -> LN2 -> FFN -> residual)
on its batch shard. Matmul operands are bf16 (fp32 PSUM accumulation); softmax,
LayerNorm statistics and the residual stream stay fp32.

Key layout choices:
  - Residual stream token-major: [128 tokens, 384] tiles, 13 tiles per octet
    (8 batches = 1600 tokens).
  - LN gains/biases folded into the weight matrices host-side (exact).
  - Activations transposed to feature-major via DMA xbar transpose (bf16).
  - Attention computed as scoresT = K^T-slices @ Q (keys on partitions), so
    softmax denominators come from ones-vector matmuls and the attV matmul
    consumes exp(scores) directly - no on-chip transpose of the weights.
  - Causal mask applied as a 0/1 multiply after exp (exp is monotone-safe
    here: |scores| <= ~3).
"""

import numpy as np
import ml_dtypes

B, T, E, F, NH, HS = 256, 200, 384, 1536, 6, 64
NCORES = 8
BPC = B // NCORES          # batches per core = 32
G = 8                      # batches per octet
NOCT = BPC // G            # 4
TOK = G * T                # 1600 tokens per octet
NT = 13                    # token tiles per octet: 12x128 + 1x64
TW = [128] * 12 + [64]     # tile widths
NCH = 4                    # 400-wide column chunks of TOK
CH = TOK // NCH            # 400

_CACHE = {}


def _install_drain_patch():
    """walrus in this container allows only one sem wait on a Drain; split the
    TileContext exit drain into a chain of single-wait drains."""
    import concourse.tile as tile
    import bass_rust
    from concourse.vector_clock import ScopedClock

    if getattr(tile.TileContext, "_drain_patch", False):
        return

    def _patched(self, tick_clock, wait_clock):
        nc = self.nc
        drain_inst = nc.sync.drain()
        wait_clock.add_sem_waits(
            drain_inst.ins, ScopedClock({None: tick_clock.global_clock})
        )
        si = drain_inst.ins.sync_info
        waits = list(si.on_wait) if si is not None else []
        if len(waits) > 1:
            si.on_wait = waits[:1]
            drain_inst.ins.sync_info = si
            for w in waits[1:]:
                d2 = nc.sync.drain()
                d2.ins.sync_info = bass_rust.SyncInfo(on_wait=[w], on_update=[])
        nc.all_engine_barrier()
        assert self.sems is not None
        popped = nc._tile_sem_poison_stack.pop()
        assert popped is self._sem_poison
        nc.clear_and_free_semaphores(list(self.sems.allocated().values()))
        nc.all_engine_barrier()

    tile.TileContext._drain_and_barrier = _patched
    tile.TileContext._drain_patch = True


def _install_wait_split_patch():
    """walrus here supports only one sync-wait per instruction on several
    templates. Split any multi-wait instruction at the BIR-JSON level into a
    chain of single-wait Drain instructions on the same engine, inserted
    immediately before it."""
    import json
    import concourse.bass_utils as bu
    import concourse.bass2jax as b2j

    if getattr(bu, "_wait_split_patch", False):
        return
    orig = bu.compile_bir_kernel

    def patched(bir_json, tmpdir, neff_name="file.neff"):
        d = json.loads(bir_json)
        uid = [0]
        for fn in d.get("functions", []):
            for bb in fn.get("blocks", []):
                new_insts = []
                for ins in bb.get("instructions", []):
                    si = ins.get("sync_info") or {}
                    waits = si.get("on_wait") or []
                    if len(waits) > 1:
                        for w in waits[:-1]:
                            uid[0] += 1
                            new_insts.append({
                                "debug": ins.get("debug", 0),
                                "engine": ins["engine"],
                                "ins": [],
                                "outs": [],
                                "is_reset_sema": False,
                                "name": f"WSPLIT-{uid[0]}",
                                "opcode": "Drain",
                                "sync_info": {"on_update": [],
                                              "on_wait": [w]},
                            })
                        si["on_wait"] = [waits[-1]]
                        ins["sync_info"] = si
                    new_insts.append(ins)
                bb["instructions"] = new_insts
        return orig(json.dumps(d).encode(), tmpdir, neff_name=neff_name)

    bu.compile_bir_kernel = patched
    b2j.compile_bir_kernel = patched
    bu._wait_split_patch = True


def _build_nc(n_octets=NOCT, stage=99, loop_reps=None):
    import concourse.bass as bass
    import concourse.mybir as mybir
    import concourse.tile as tile

    _install_drain_patch()
    f32 = mybir.dt.float32
    bf16 = mybir.dt.bfloat16
    AF = mybir.ActivationFunctionType
    OP = mybir.AluOpType

    nc = bass.Bass("TRN2")

    x_d = nc.dram_tensor("x", [BPC, T, E], f32, kind="ExternalInput")
    wq_d = nc.dram_tensor("wq", [E, E], bf16, kind="ExternalInput")
    wk_d = nc.dram_tensor("wk", [E, E], bf16, kind="ExternalInput")
    wv_d = nc.dram_tensor("wv", [E, E], bf16, kind="ExternalInput")
    wp_d = nc.dram_tensor("wp", [E, E], bf16, kind="ExternalInput")
    w1_d = nc.dram_tensor("w1", [E, F], bf16, kind="ExternalInput")
    w2_d = nc.dram_tensor("w2", [F, E], bf16, kind="ExternalInput")
    cq_d = nc.dram_tensor("cq", [E], f32, kind="ExternalInput")
    ck_d = nc.dram_tensor("ck", [E], f32, kind="ExternalInput")
    b1_d = nc.dram_tensor("b1p", [F], f32, kind="ExternalInput")
    bp_d = nc.dram_tensor("bpb", [1, E], bf16, kind="ExternalInput")
    b2_d = nc.dram_tensor("b2b", [1, E], bf16, kind="ExternalInput")
    m0_d = nc.dram_tensor("m0", [128, NH, T], bf16, kind="ExternalInput")
    m1_d = nc.dram_tensor("m1", [72, NH, 72], bf16, kind="ExternalInput")
    oc_d = nc.dram_tensor("onc", [128, 1], bf16, kind="ExternalInput")
    or_d = nc.dram_tensor("onr", [1, 128], bf16, kind="ExternalInput")
    it0_d = nc.dram_tensor("ind0", [1, 128], f32, kind="ExternalInput")
    it1_d = nc.dram_tensor("ind1", [1, 128], f32, kind="ExternalInput")
    y_d = nc.dram_tensor("y", [BPC, T, E], f32, kind="ExternalOutput")

    x_flat = x_d[:].rearrange("b t d -> (b t) d")
    y_flat = y_d[:].rearrange("b t d -> (b t) d")

    from contextlib import ExitStack

    with tile.TileContext(nc) as tc, ExitStack() as es:
        cpool = es.enter_context(tc.tile_pool(name="const", bufs=1))
        spool = es.enter_context(tc.tile_pool(name="work", bufs=1))
        # small rotating pools
        hpool = es.enter_context(tc.tile_pool(name="hot", bufs=2))
        opool = es.enter_context(tc.tile_pool(name="out", bufs=3))
        ppool = es.enter_context(tc.tile_pool(name="ps", bufs=4, space="PSUM"))
        p1pool = es.enter_context(tc.tile_pool(name="ps1", bufs=1, space="PSUM"))
        papool = es.enter_context(tc.tile_pool(name="pa", bufs=1, space="PSUM"))

        # ---- constants ----
        wq_s = cpool.tile([128, 3, E], bf16, tag="wq")
        wk_s = cpool.tile([128, 3, E], bf16, tag="wk")
        wv_s = cpool.tile([128, 3, E], bf16, tag="wv")
        wp_s = cpool.tile([128, 3, E], bf16, tag="wp")
        w1_s = cpool.tile([128, 3, F], bf16, tag="w1")
        w2_s = cpool.tile([128, 12, E], bf16, tag="w2")
        for dst, src in ((wq_s, wq_d), (wk_s, wk_d), (wv_s, wv_d), (wp_s, wp_d),
                         (w1_s, w1_d), (w2_s, w2_d)):
            nc.sync.dma_start(dst[:], src[:].rearrange("(ko p) m -> p ko m", p=128))
        cq_s = cpool.tile([128, 3], f32, tag="cq")
        ck_s = cpool.tile([128, 3], f32, tag="ck")
        b1_s = cpool.tile([128, 12], f32, tag="b1")
        nc.sync.dma_start(cq_s[:], cq_d[:].rearrange("(mo p) -> p mo", p=128))
        nc.sync.dma_start(ck_s[:], ck_d[:].rearrange("(mo p) -> p mo", p=128))
        nc.sync.dma_start(b1_s[:], b1_d[:].rearrange("(mo p) -> p mo", p=128))
        bp_s = cpool.tile([1, E], bf16, tag="bp")
        b2_s = cpool.tile([1, E], bf16, tag="b2")
        nc.sync.dma_start(bp_s[:], bp_d[:])
        nc.sync.dma_start(b2_s[:], b2_d[:])
        m0_s = cpool.tile([128, NH, T], bf16, tag="m0")
        m1_s = cpool.tile([72, NH, 72], bf16, tag="m1")
        oc_s = cpool.tile([128, 1], bf16, tag="onc")
        or_s = cpool.tile([1, 128], bf16, tag="onr")
        it0_s = cpool.tile([1, 128], f32, tag="ind0")
        it1_s = cpool.tile([1, 128], f32, tag="ind1")
        eps_s = cpool.tile([128, 1], f32, tag="eps")
        nc.vector.memset(eps_s[:], 1e-5)
        nc.sync.dma_start(m0_s[:], m0_d[:])
        nc.sync.dma_start(m1_s[:], m1_d[:])
        nc.sync.dma_start(oc_s[:], oc_d[:])
        nc.sync.dma_start(or_s[:], or_d[:])
        nc.sync.dma_start(it0_s[:], it0_d[:])
        nc.sync.dma_start(it1_s[:], it1_d[:])

        def layernorm(src_tile, dst_tile):
            """src [128, NT, E] f32 -> dst [128, NT, E] bf16 (normalized, no
            gain/bias - folded into weights)."""
            stats = spool.tile([128, NT, 6], f32, tag="stats")
            mv = spool.tile([128, NT, 2], f32, tag="mv")
            nc.vector.memset(mv[:], 1.0)
            for i in range(NT):
                w = TW[i]
                nc.vector.bn_stats(stats[:w, i, :], src_tile[:w, i, :])
            for i in range(NT):
                w = TW[i]
                nc.vector.bn_aggr(mv[:w, i, :], stats[:w, i, :])
            sd = spool.tile([128, NT], f32, tag="sd")
            av = spool.tile([128, NT], f32, tag="av")
            b0 = spool.tile([128, NT], f32, tag="b0")
            nc.scalar.activation(sd[:], mv[:, :, 1], AF.Sqrt, bias=eps_s[:, 0:1])
            nc.vector.reciprocal(av[:], sd[:])
            nc.vector.tensor_tensor(b0[:], mv[:, :, 0], av[:], OP.mult)
            nc.vector.tensor_scalar(b0[:], b0[:], -1.0, None, OP.mult)
            for i in range(NT):
                w = TW[i]
                nc.vector.tensor_scalar(
                    dst_tile[:w, i, :], src_tile[:w, i, :],
                    av[:w, i : i + 1], b0[:w, i : i + 1], OP.mult, OP.add,
                )

        def transpose_feat(src_tile, dst_tile):
            """src [128, NT, E] bf16 token-major -> dst [128, 3, TOK] bf16
            feature-major, via xbar transpose of [w,128] blocks."""
            for i in range(NT):
                w = TW[i]
                for k in range(3):
                    nc.sync.dma_start_transpose(
                        dst_tile[:, k, 128 * i : 128 * i + w],
                        src_tile[:w, i, 128 * k : 128 * (k + 1)],
                    )

        def dump(tile_ap, nrows, row0, ncols=E):
            # cast tile [p, free] to f32 and write into y rows row0..row0+nrows
            d = opool.tile([128, E], f32, tag="ot")
            nc.vector.tensor_copy(d[:nrows, :ncols], tile_ap)
            nc.sync.dma_start(y_flat[row0 : row0 + nrows], d[:nrows, :])

        octet_range = range(n_octets)
        loop_cm = None
        if loop_reps is not None:
            loop_cm = tc.For_i(0, loop_reps, 1)
            loop_cm.__enter__()
        for o in octet_range:
            r0 = o * TOK
            x_oct = spool.tile([128, NT, E], f32, tag="resid")
            nc.sync.dma_start(
                x_oct[:, 0:12, :],
                x_flat[r0 : r0 + 1536].rearrange("(g p) d -> p g d", p=128),
            )
            nc.sync.dma_start(x_oct[0:64, 12, :], x_flat[r0 + 1536 : r0 + 1600])

            if stage <= 1:
                for i in range(NT):
                    w = TW[i]
                    dump(x_oct[:w, i, :], w, r0 + 128 * i)
                continue

            # ---- LN1 ----
            h_all = spool.tile([128, NT, E], bf16, tag="h")
            layernorm(x_oct, h_all)

            if stage <= 2:
                for i in range(NT):
                    w = TW[i]
                    dump(h_all[:w, i, :], w, r0 + 128 * i)
                continue

            # ---- transpose h -> hT ----
            hT = spool.tile([128, 3, TOK], bf16, tag="hT")
            transpose_feat(h_all, hT)

            # ---- qT/kT (feature-major, weight-stationary) ----
            qT = spool.tile([128, 3, TOK], bf16, tag="qT")
            kT = spool.tile([128, 3, TOK], bf16, tag="kT")
            for dstT, w_s, c_s in ((qT, wq_s, cq_s), (kT, wk_s, ck_s)):
                for m in range(3):
                    for c in range(NCH):
                        pq = ppool.tile([128, CH], f32, tag="b1")
                        for k in range(3):
                            nc.tensor.matmul(
                                pq[:],
                                w_s[:, k, 128 * m : 128 * (m + 1)],
                                hT[:, k, CH * c : CH * (c + 1)],
                                start=(k == 0), stop=(k == 2),
                            )
                        nc.scalar.activation(
                            dstT[:, m, CH * c : CH * (c + 1)], pq[:],
                            AF.Identity, bias=c_s[:, m : m + 1],
                        )

            if stage <= 3:
                for i in range(4):
                    dump(kT[:, 0, 384 * i : 384 * (i + 1)], 128, r0 + 128 * i)
                continue

            # ---- v (token-major, batch-aligned tiles) ----
            v_all = spool.tile([128, G, 2, E], bf16, tag="v")
            for b in range(G):
                for tt in range(2):
                    w = 128 if tt == 0 else 72
                    col = 200 * b + 128 * tt
                    pv = ppool.tile([128, E], f32, tag="b1")
                    for k in range(3):
                        nc.tensor.matmul(
                            pv[:w, :],
                            hT[:, k, col : col + w],
                            wv_s[:, k, :],
                            start=(k == 0), stop=(k == 2),
                        )
                    nc.scalar.activation(v_all[:w, b, tt, :], pv[:w, :], AF.Copy)

            if stage <= 4:
                for i in range(4):
                    dump(v_all[:, i, 0, 0:384], 128, r0 + 128 * i)
                continue

            # ---- attention + proj ----
            attT = None
            if stage > 45:
                attT = spool.tile([128, 3, TOK], bf16, tag="attT", name="attT")
            for b in range(G):
                c0 = 200 * b
                expT0 = hpool.tile([128, NH, T], bf16, tag="expT0")
                expT1 = hpool.tile([72, NH, 72], bf16, tag="expT1")
                # matmul operands must start at partition 0: stage the odd
                # heads' rows (partitions 64-127) of qT/kT down to base 0.
                qstg = hpool.tile([64, 3, T], bf16, tag="qstg")
                kstg = hpool.tile([64, 3, T], bf16, tag="kstg")
                nc.sync.dma_start(qstg[:], qT[64:128, :, c0 : c0 + T])
                nc.sync.dma_start(kstg[:], kT[64:128, :, c0 : c0 + T])

                def kslice(j, r, lo, hi):
                    if r == 0:
                        return kT[0:64, j, c0 + lo : c0 + hi]
                    return kstg[:, j, lo:hi]

                def qslice(j, r, lo, hi):
                    if r == 0:
                        return qT[0:64, j, c0 + lo : c0 + hi]
                    return qstg[:, j, lo:hi]

                ps_list = []
                for j in range(3):
                    ps = ppool.tile([128, 2, T], f32, tag="b1")
                    ps_list.append(ps)
                    for r in range(2):
                        nc.tensor.matmul(
                            ps[:, r, :],
                            kslice(j, r, 0, 128),
                            qslice(j, r, 0, T),
                            start=True, stop=True,
                        )
                ps1 = p1pool.tile([72, NH, 72], f32, tag="ps1")
                for j in range(3):
                    for r in range(2):
                        h = 2 * j + r
                        nc.tensor.matmul(
                            ps1[:, h, :],
                            kslice(j, r, 128, 200),
                            qslice(j, r, 128, 200),
                            start=True, stop=True,
                        )
                for j in range(3):
                    nc.scalar.activation(
                        expT0[:, 2 * j : 2 * j + 2, :], ps_list[j][:], AF.Exp
                    )
                nc.scalar.activation(expT1[:], ps1[:], AF.Exp)
                if stage <= 41:
                    dump(expT0[:, 0, :], 128, r0 + 200 * b, ncols=T)
                    continue
                nc.vector.tensor_tensor(expT0[:], expT0[:], m0_s[:], OP.mult)
                nc.vector.tensor_tensor(expT1[:], expT1[:], m1_s[:], OP.mult)
                if stage <= 42:
                    dump(expT0[:, 0, :], 128, r0 + 200 * b, ncols=T)
                    continue

                # denominators: ones-matmul -> [1, 2, T] per head pair
                recip = hpool.tile([1, NH, T], f32, tag="recip")
                for j in range(3):
                    sm = ppool.tile([1, 2, T], f32, tag="b1")
                    nc.tensor.matmul(
                        sm[:], oc_s[:, :], expT0[:, 2 * j : 2 * j + 2, :],
                        start=True, stop=False,
                    )
                    for r in range(2):
                        nc.tensor.matmul(
                            sm[:, r, 128:200], oc_s[0:72, :],
                            expT1[:, 2 * j + r, :],
                            start=False, stop=(r == 1),
                        )
                    nc.vector.reciprocal(recip[0:1, 2 * j : 2 * j + 2, :], sm[:])
                if stage <= 43:
                    dump(recip[0:1, 0, :], 1, r0 + 200 * b, ncols=T)
                    continue

                # attV (unnormalized) into pa banks
                pa = papool.tile([128, 3, 512], f32, tag="pa")
                for j in range(3):
                    for r in range(2):
                        h = 2 * j + r
                        nc.tensor.matmul(
                            pa[64 * r : 64 * r + 64, j, 0:T],
                            v_all[0:128, b, 0, 64 * h : 64 * h + 64],
                            expT0[:, h, :],
                            start=True, stop=False,
                        )
                        nc.tensor.matmul(
                            pa[64 * r : 64 * r + 64, j, 128:200],
                            v_all[0:72, b, 1, 64 * h : 64 * h + 64],
                            expT1[:, h, :],
                            start=False, stop=True,
                        )

                if stage <= 44:
                    dump(pa[:, 0, 0:384], 128, r0 + 200 * b)
                    continue

                # broadcast recip across partitions (K=1 matmuls), normalize
                for j in range(3):
                    rb = ppool.tile([128, T], f32, tag="b1")
                    nc.tensor.matmul(
                        rb[:], it0_s[:], recip[0:1, 2 * j, :],
                        start=True, stop=False,
                    )
                    nc.tensor.matmul(
                        rb[:], it1_s[:], recip[0:1, 2 * j + 1, :],
                        start=False, stop=True,
                    )
                    rbs = hpool.tile([128, T], f32, tag="rbs")
                    nc.scalar.activation(rbs[:], rb[:], AF.Copy)
                    if stage <= 45:
                        continue
                    nc.vector.tensor_tensor(
                        attT[:, j, c0 : c0 + T], pa[:, j, 0:T], rbs[:], OP.mult
                    )
                if stage <= 45:
                    dump(pa[:, 0, 0:384], 128, r0 + 200 * b)
                    continue

            if stage == 5 or (41 <= stage <= 45):
                if stage == 5:
                    for i in range(4):
                        dump(attT[:, 0, 384 * i : 384 * (i + 1)], 128, r0 + 128 * i)
                continue

            # ---- proj + residual ----
            x1 = spool.tile([128, NT, E], f32, tag="resid2")
            for i in range(NT):
                w = TW[i]
                pp = ppool.tile([128, E], f32, tag="b1")
                for k in range(3):
                    nc.tensor.matmul(
                        pp[:w, :],
                        attT[:, k, 128 * i : 128 * i + w],
                        wp_s[:, k, :],
                        start=(k == 0), stop=False,
                    )
                nc.tensor.matmul(
                    pp[:w, :], or_s[0:1, 0:w], bp_s[:],
                    start=False, stop=True,
                )
                nc.vector.tensor_tensor(
                    x1[:w, i, :], x_oct[:w, i, :], pp[:w, :], OP.add
                )

            if stage <= 6:
                for i in range(NT):
                    w = TW[i]
                    dump(x1[:w, i, :], w, r0 + 128 * i)
                continue

            # ---- LN2 + transpose ----
            h2 = spool.tile([128, NT, E], bf16, tag="h")
            layernorm(x1, h2)
            h2T = spool.tile([128, 3, TOK], bf16, tag="hT")
            transpose_feat(h2, h2T)

            # ---- FFN1 + ReLU ----
            uT = spool.tile([128, 12, TOK], bf16, tag="uT")
            for m in range(12):
                for c in range(NCH):
                    pu = ppool.tile([128, CH], f32, tag="b1")
                    for k in range(3):
                        nc.tensor.matmul(
                            pu[:],
                            w1_s[:, k, 128 * m : 128 * (m + 1)],
                            h2T[:, k, CH * c : CH * (c + 1)],
                            start=(k == 0), stop=(k == 2),
                        )
                    nc.scalar.activation(
                        uT[:, m, CH * c : CH * (c + 1)], pu[:],
                        AF.Relu, bias=b1_s[:, m : m + 1],
                    )

            if stage <= 7:
                for i in range(4):
                    dump(uT[:, 0, 384 * i : 384 * (i + 1)], 128, r0 + 128 * i)
                continue

            # ---- FFN2 + residual + store ----
            for i in range(NT):
                w = TW[i]
                pf = ppool.tile([128, E], f32, tag="b1")
                for k in range(12):
                    nc.tensor.matmul(
                        pf[:w, :],
                        uT[:, k, 128 * i : 128 * i + w],
                        w2_s[:, k, :],
                        start=(k == 0), stop=False,
                    )
                nc.tensor.matmul(
                    pf[:w, :], or_s[0:1, 0:w], b2_s[:],
                    start=False, stop=True,
                )
                ot = opool.tile([128, E], f32, tag="ot")
                nc.vector.tensor_tensor(
                    ot[:w, :], x1[:w, i, :], pf[:w, :], OP.add
                )
                nc.sync.dma_start(y_flat[r0 + 128 * i : r0 + 128 * i + w], ot[:w, :])

        if loop_cm is not None:
            loop_cm.__exit__(None, None, None)

    return nc


def _prep_inputs(inputs):
    """Host-side folding of LN gains/biases into weights. Exact in fp32."""
    bf = ml_dtypes.bfloat16
    x = np.asarray(inputs["x"], np.float32)
    Wq = np.asarray(inputs["Wq"], np.float32)
    Wk = np.asarray(inputs["Wk"], np.float32)
    Wv = np.asarray(inputs["Wv"], np.float32)
    Wp = np.asarray(inputs["Wproj"], np.float32)
    bproj = np.asarray(inputs["bproj"], np.float32)
    W1 = np.asarray(inputs["W1"], np.float32)
    b1 = np.asarray(inputs["b1"], np.float32)
    W2 = np.asarray(inputs["W2"], np.float32)
    b2 = np.asarray(inputs["b2"], np.float32)
    g1 = np.asarray(inputs["g1"], np.float32)
    be1 = np.asarray(inputs["be1"], np.float32)
    g2 = np.asarray(inputs["g2"], np.float32)
    be2 = np.asarray(inputs["be2"], np.float32)

    s = E ** -0.5
    wq_f = (g1[:, None] * Wq) * s
    wk_f = g1[:, None] * Wk
    wv_f = g1[:, None] * Wv
    cq = (be1 @ Wq) * s
    ck = be1 @ Wk
    cv = be1 @ Wv
    bp_f = bproj + cv @ Wp
    w1_f = g2[:, None] * W1
    b1_f = b1 + be2 @ W1

    m0 = np.zeros((128, NH, T), np.float32)
    sidx = np.arange(128)[:, None]
    tidx = np.arange(T)[None, :]
    m0[:, :, :] = (tidx >= sidx)[:, None, :]
    m1 = np.zeros((72, NH, 72), np.float32)
    si = np.arange(72)[:, None]
    ti = np.arange(72)[None, :]
    m1[:, :, :] = (ti >= si)[:, None, :]

    ind0 = np.zeros((1, 128), np.float32); ind0[0, 0:64] = 1.0
    ind1 = np.zeros((1, 128), np.float32); ind1[0, 64:128] = 1.0

    common = {
        "wq": wq_f.astype(bf), "wk": wk_f.astype(bf), "wv": wv_f.astype(bf),
        "wp": Wp.astype(bf), "w1": w1_f.astype(bf), "w2": W2.astype(bf),
        "cq": cq, "ck": ck, "b1p": b1_f,
        "bpb": bp_f.astype(bf).reshape(1, E), "b2b": b2.astype(bf).reshape(1, E),
        "m0": m0.astype(bf), "m1": m1.astype(bf),
        "onc": np.ones((128, 1), bf), "onr": np.ones((1, 128), bf),
        "ind0": ind0, "ind1": ind1,
    }
    return x, common


def kernel(**inputs):
    from concourse.bass_utils import run_bass_kernel_spmd

    _install_wait_split_patch()

    x, common = _prep_inputs(inputs)
    if "nc" not in _CACHE:
        _CACHE["nc"] = _build_nc()
    nc = _CACHE["nc"]
    in_maps = []
    for c in range(NCORES):
        m = dict(common)
        m["x"] = np.ascontiguousarray(x[c * BPC : (c + 1) * BPC])
        in_maps.append(m)
    res = run_bass_kernel_spmd(nc, in_maps, core_ids=list(range(NCORES)))
    out = np.concatenate([res.results[c]["y"] for c in range(NCORES)], axis=0)
    return out.astype(np.float32)



# revision 2
# speedup vs baseline: 2.2485x; 2.2485x over previous
"""Trainium2 Bass kernel for a pre-LN transformer block (B=256, T=200, E=384).

Data-parallel over batch: 8 NeuronCores x 32 batches. Each core runs the full
block (LN1 -> QKV -> causal attention -> proj+residual -> LN2 -> FFN -> residual)
on its batch shard. Matmul operands are bf16 (fp32 PSUM accumulation); softmax,
LayerNorm statistics and the residual stream stay fp32.

Key layout choices:
  - Residual stream token-major: [128 tokens, 384] tiles, 13 tiles per octet
    (8 batches = 1600 tokens), updated in place by both residual adds.
  - LN gains/biases folded into the weight matrices host-side (exact).
  - LN inv-std via DVE bit-trick rsqrt + 2 Newton steps (keeps ScalarE on the
    exp/copy/relu activation table - no LUT reloads).
  - Activations transposed to feature-major via DMA xbar transpose (bf16).
  - Attention: scoresT = K^T-slices @ Q with keys on partitions; odd heads read
    directly from partitions 64-127 via matmul tile_position (no staging).
  - Softmax denominators via column-mask ones matmuls that land broadcast
    across partitions in head-interleaved form; reciprocal_approx_fast on DVE;
    one fused normalize multiply per batch.
  - Causal mask applied as a 0/1 multiply after exp on GpSimd (exp is
    monotone-safe here: |scores| <= ~3).
"""

import numpy as np
import ml_dtypes

B, T, E, F, NH, HS = 256, 200, 384, 1536, 6, 64
NCORES = 8
BPC = B // NCORES          # batches per core = 32
G = 8                      # batches per octet
NOCT = BPC // G            # 4
TOK = G * T                # 1600 tokens per octet
NT = 13                    # token tiles per octet: 12x128 + 1x64
TW = [128] * 12 + [64]     # tile widths
NCH = 4                    # 400-wide column chunks of TOK
CH = TOK // NCH            # 400

_CACHE = {}


def _install_drain_patch():
    """walrus in this container allows only one sem wait on a Drain; split the
    TileContext exit drain into a chain of single-wait drains."""
    import concourse.tile as tile
    import bass_rust
    from concourse.vector_clock import ScopedClock

    if getattr(tile.TileContext, "_drain_patch", False):
        return

    def _patched(self, tick_clock, wait_clock):
        nc = self.nc
        drain_inst = nc.sync.drain()
        wait_clock.add_sem_waits(
            drain_inst.ins, ScopedClock({None: tick_clock.global_clock})
        )
        si = drain_inst.ins.sync_info
        waits = list(si.on_wait) if si is not None else []
        if len(waits) > 1:
            si.on_wait = waits[:1]
            drain_inst.ins.sync_info = si
            for w in waits[1:]:
                d2 = nc.sync.drain()
                d2.ins.sync_info = bass_rust.SyncInfo(on_wait=[w], on_update=[])
        nc.all_engine_barrier()
        assert self.sems is not None
        popped = nc._tile_sem_poison_stack.pop()
        assert popped is self._sem_poison
        nc.clear_and_free_semaphores(list(self.sems.allocated().values()))
        nc.all_engine_barrier()

    tile.TileContext._drain_and_barrier = _patched
    tile.TileContext._drain_patch = True


def _install_wait_split_patch():
    """walrus here supports only one sync-wait per instruction on several
    templates. Split any multi-wait instruction at the BIR-JSON level into a
    chain of single-wait Drain instructions on the same engine, inserted
    immediately before it."""
    import json
    import concourse.bass_utils as bu
    import concourse.bass2jax as b2j

    if getattr(bu, "_wait_split_patch", False):
        return
    orig = bu.compile_bir_kernel

    def patched(bir_json, tmpdir, neff_name="file.neff"):
        d = json.loads(bir_json)
        uid = [0]
        for fn in d.get("functions", []):
            for bb in fn.get("blocks", []):
                new_insts = []
                for ins in bb.get("instructions", []):
                    si = ins.get("sync_info") or {}
                    waits = si.get("on_wait") or []
                    if len(waits) > 1:
                        for w in waits[:-1]:
                            uid[0] += 1
                            new_insts.append({
                                "debug": ins.get("debug", 0),
                                "engine": ins["engine"],
                                "ins": [],
                                "outs": [],
                                "is_reset_sema": False,
                                "name": f"WSPLIT-{uid[0]}",
                                "opcode": "Drain",
                                "sync_info": {"on_update": [],
                                              "on_wait": [w]},
                            })
                        si["on_wait"] = [waits[-1]]
                        ins["sync_info"] = si
                    new_insts.append(ins)
                bb["instructions"] = new_insts
        return orig(json.dumps(d).encode(), tmpdir, neff_name=neff_name)

    bu.compile_bir_kernel = patched
    b2j.compile_bir_kernel = patched
    bu._wait_split_patch = True


RSQRT_MAGIC = 0x5F3759DF


def _build_nc(n_octets=NOCT, loop_reps=None):
    import concourse.bass as bass
    import concourse.mybir as mybir
    import concourse.tile as tile

    _install_drain_patch()
    f32 = mybir.dt.float32
    i32 = mybir.dt.int32
    bf16 = mybir.dt.bfloat16
    AF = mybir.ActivationFunctionType
    OP = mybir.AluOpType

    nc = bass.Bass("TRN2")

    x_d = nc.dram_tensor("x", [BPC, T, E], f32, kind="ExternalInput")
    wq_d = nc.dram_tensor("wq", [E, E], bf16, kind="ExternalInput")
    wk_d = nc.dram_tensor("wk", [E, E], bf16, kind="ExternalInput")
    wv_d = nc.dram_tensor("wv", [E, E], bf16, kind="ExternalInput")
    wp_d = nc.dram_tensor("wp", [E, E], bf16, kind="ExternalInput")
    w1_d = nc.dram_tensor("w1", [E, F], bf16, kind="ExternalInput")
    w2_d = nc.dram_tensor("w2", [F, E], bf16, kind="ExternalInput")
    cq_d = nc.dram_tensor("cq", [E], f32, kind="ExternalInput")
    ck_d = nc.dram_tensor("ck", [E], f32, kind="ExternalInput")
    b1_d = nc.dram_tensor("b1p", [F], f32, kind="ExternalInput")
    bp_d = nc.dram_tensor("bpb", [1, E], bf16, kind="ExternalInput")
    b2_d = nc.dram_tensor("b2b", [1, E], bf16, kind="ExternalInput")
    m0_d = nc.dram_tensor("m0", [128, NH, T], bf16, kind="ExternalInput")
    m1_d = nc.dram_tensor("m1", [72, NH, 72], bf16, kind="ExternalInput")
    or_d = nc.dram_tensor("onr", [1, 128], bf16, kind="ExternalInput")
    y_d = nc.dram_tensor("y", [BPC, T, E], f32, kind="ExternalOutput")

    x_flat = x_d[:].rearrange("b t d -> (b t) d")
    y_flat = y_d[:].rearrange("b t d -> (b t) d")

    from contextlib import ExitStack

    with tile.TileContext(nc) as tc, ExitStack() as es:
        cpool = es.enter_context(tc.tile_pool(name="const", bufs=1))
        spool = es.enter_context(tc.tile_pool(name="work", bufs=1))
        dpool = es.enter_context(tc.tile_pool(name="dbuf", bufs=2))
        hpool = es.enter_context(tc.tile_pool(name="hot", bufs=2))
        mmpool = es.enter_context(tc.tile_pool(name="mm", bufs=3, space="PSUM"))
        papool = es.enter_context(tc.tile_pool(name="pa", bufs=2, space="PSUM"))
        p1pool = es.enter_context(tc.tile_pool(name="ps1", bufs=1, space="PSUM"))

        # ---- constants ----
        wq_s = cpool.tile([128, 3, E], bf16, tag="wq")
        wk_s = cpool.tile([128, 3, E], bf16, tag="wk")
        wv_s = cpool.tile([128, 3, E], bf16, tag="wv")
        wp_s = cpool.tile([128, 3, E], bf16, tag="wp")
        w1_s = cpool.tile([128, 3, F], bf16, tag="w1")
        w2_s = cpool.tile([128, 12, E], bf16, tag="w2")
        for dst, src in ((wq_s, wq_d), (wk_s, wk_d), (wv_s, wv_d), (wp_s, wp_d),
                         (w1_s, w1_d), (w2_s, w2_d)):
            nc.sync.dma_start(dst[:], src[:].rearrange("(ko p) m -> p ko m", p=128))
        cq_s = cpool.tile([128, 3], f32, tag="cq")
        ck_s = cpool.tile([128, 3], f32, tag="ck")
        b1_s = cpool.tile([128, 12], f32, tag="b1")
        nc.sync.dma_start(cq_s[:], cq_d[:].rearrange("(mo p) -> p mo", p=128))
        nc.sync.dma_start(ck_s[:], ck_d[:].rearrange("(mo p) -> p mo", p=128))
        nc.sync.dma_start(b1_s[:], b1_d[:].rearrange("(mo p) -> p mo", p=128))
        bp_s = cpool.tile([1, E], bf16, tag="bp")
        b2_s = cpool.tile([1, E], bf16, tag="b2")
        nc.sync.dma_start(bp_s[:], bp_d[:])
        nc.sync.dma_start(b2_s[:], b2_d[:])
        m0_s = cpool.tile([128, NH, T], bf16, tag="m0")
        m1_s = cpool.tile([72, NH, 72], bf16, tag="m1")
        or_s = cpool.tile([1, 128], bf16, tag="onr")
        nc.sync.dma_start(m0_s[:], m0_d[:])
        nc.sync.dma_start(m1_s[:], m1_d[:])
        nc.sync.dma_start(or_s[:], or_d[:])
        # column-half masks for interleaved softmax denominators
        cmL = cpool.tile([128, 128], bf16, tag="cmL")
        cmR = cpool.tile([128, 128], bf16, tag="cmR")
        nc.vector.memset(cmL[:], 0.0)
        nc.vector.memset(cmL[:, 0:64], 1.0)
        nc.vector.memset(cmR[:], 0.0)
        nc.vector.memset(cmR[:, 64:128], 1.0)

        def layernorm(src_tile, dst_tile):
            """src [128, NT, E] f32 -> dst [128, NT, E] bf16 (normalized, no
            gain/bias - folded into weights). inv-std on DVE (bit-trick rsqrt
            + 2 Newton steps) - keeps ScalarE's LUT on the exp table."""
            stats = spool.tile([128, NT, 6], f32, tag="stats")
            mv = spool.tile([128, NT, 2], f32, tag="mv")
            nc.vector.memset(mv[:], 1.0)
            for i in range(NT):
                w = TW[i]
                nc.vector.bn_stats(stats[:w, i, :], src_tile[:w, i, :])
            for i in range(NT):
                w = TW[i]
                nc.vector.bn_aggr(mv[:w, i, :], stats[:w, i, :])
            t = spool.tile([128, NT], f32, tag="lt")
            y0 = spool.tile([128, NT], f32, tag="ly0")
            p = spool.tile([128, NT], f32, tag="lp")
            r = spool.tile([128, NT], f32, tag="lr")
            av = spool.tile([128, NT], f32, tag="av")
            b0 = spool.tile([128, NT], f32, tag="b0")
            nc.vector.tensor_scalar(t[:], mv[:, :, 1], 1e-5, None, OP.add)
            # seed: y0 = bitcast(MAGIC + ((~i) >> 1)) ~= rsqrt(t)
            nc.vector.tensor_scalar(
                y0[:].bitcast(i32), t[:].bitcast(i32),
                -1, 1, OP.bitwise_xor, OP.arith_shift_right,
            )
            nc.vector.tensor_scalar(
                y0[:].bitcast(i32), y0[:].bitcast(i32),
                RSQRT_MAGIC, None, OP.add,
            )
            for dst in (r, av):  # 2 Newton steps: y <- y*(1.5 - 0.5*t*y^2)
                nc.vector.tensor_tensor(p[:], t[:], y0[:], OP.mult)
                nc.vector.tensor_tensor(p[:], p[:], y0[:], OP.mult)
                nc.vector.tensor_scalar(p[:], p[:], -0.5, 1.5, OP.mult, OP.add)
                nc.vector.tensor_tensor(dst[:], y0[:], p[:], OP.mult)
                y0 = dst
            nc.vector.tensor_tensor(b0[:], mv[:, :, 0], av[:], OP.mult)
            nc.vector.tensor_scalar(b0[:], b0[:], -1.0, None, OP.mult)
            for i in range(NT):
                w = TW[i]
                nc.vector.tensor_scalar(
                    dst_tile[:w, i, :], src_tile[:w, i, :],
                    av[:w, i : i + 1], b0[:w, i : i + 1], OP.mult, OP.add,
                )

        def transpose_feat(src_tile, dst_tile):
            """src [128, NT, E] bf16 token-major -> dst [128, 3, TOK] bf16
            feature-major, via xbar transpose of [w,128] blocks."""
            for i in range(NT):
                w = TW[i]
                for k in range(3):
                    nc.sync.dma_start_transpose(
                        dst_tile[:, k, 128 * i : 128 * i + w],
                        src_tile[:w, i, 128 * k : 128 * (k + 1)],
                    )

        octet_range = range(n_octets)
        loop_cm = None
        if loop_reps is not None:
            loop_cm = tc.For_i(0, loop_reps, 1)
            loop_cm.__enter__()
        for o in octet_range:
            r0 = o * TOK
            x_oct = dpool.tile([128, NT, E], f32, tag="resid")
            nc.sync.dma_start(
                x_oct[:, 0:12, :],
                x_flat[r0 : r0 + 1536].rearrange("(g p) d -> p g d", p=128),
            )
            nc.sync.dma_start(x_oct[0:64, 12, :], x_flat[r0 + 1536 : r0 + 1600])

            # ---- LN1 ----
            h_all = spool.tile([128, NT, E], bf16, tag="h")
            layernorm(x_oct, h_all)

            # ---- transpose h -> hT ----
            hT = spool.tile([128, 3, TOK], bf16, tag="hT")
            transpose_feat(h_all, hT)

            # ---- qT/kT (feature-major, weight-stationary) ----
            qT = dpool.tile([128, 3, TOK], bf16, tag="qT")
            kT = dpool.tile([128, 3, TOK], bf16, tag="kT")
            for dstT, w_s, c_s in ((qT, wq_s, cq_s), (kT, wk_s, ck_s)):
                for m in range(3):
                    for c in range(NCH):
                        pq = mmpool.tile([128, CH], f32, tag="mm")
                        for k in range(3):
                            nc.tensor.matmul(
                                pq[:],
                                w_s[:, k, 128 * m : 128 * (m + 1)],
                                hT[:, k, CH * c : CH * (c + 1)],
                                start=(k == 0), stop=(k == 2),
                            )
                        nc.scalar.activation(
                            dstT[:, m, CH * c : CH * (c + 1)], pq[:],
                            AF.Identity, bias=c_s[:, m : m + 1],
                        )

            # ---- v (token-major, batch-aligned tiles) ----
            v_all = spool.tile([128, G, 2, E], bf16, tag="v")
            for b in range(G):
                for tt in range(2):
                    w = 128 if tt == 0 else 72
                    col = 200 * b + 128 * tt
                    pv = mmpool.tile([128, E], f32, tag="mm")
                    for k in range(3):
                        nc.tensor.matmul(
                            pv[:w, :],
                            hT[:, k, col : col + w],
                            wv_s[:, k, :],
                            start=(k == 0), stop=(k == 2),
                        )
                    nc.scalar.activation(v_all[:w, b, tt, :], pv[:w, :], AF.Copy)

            # ---- attention ----
            attT = spool.tile([128, 3, TOK], bf16, tag="attT")
            for b in range(G):
                c0 = 200 * b
                expT0 = hpool.tile([128, NH, T], bf16, tag="expT0")
                expT1 = hpool.tile([72, NH, 72], bf16, tag="expT1")

                def kslice(j, r, lo, hi):
                    return kT[64 * r : 64 * r + 64, j, c0 + lo : c0 + hi]

                def qslice(j, r, lo, hi):
                    return qT[64 * r : 64 * r + 64, j, c0 + lo : c0 + hi]

                ps_list = []
                for j in range(3):
                    ps = mmpool.tile([128, 2, 256], f32, tag="mm")
                    ps_list.append(ps)
                    for r in range(2):
                        nc.tensor.matmul(
                            ps[:, r, 0:T],
                            kslice(j, r, 0, 128),
                            qslice(j, r, 0, T),
                            start=True, stop=True,
                        )
                ps1 = p1pool.tile([72, NH, 72], f32, tag="ps1")
                for j in range(3):
                    for r in range(2):
                        h = 2 * j + r
                        nc.tensor.matmul(
                            ps1[:, h, :],
                            kslice(j, r, 128, 200),
                            qslice(j, r, 128, 200),
                            start=True, stop=True,
                        )
                for j in range(3):
                    nc.scalar.activation(
                        expT0[:, 2 * j : 2 * j + 2, :], ps_list[j][:, :, 0:T],
                        AF.Exp,
                    )
                nc.scalar.activation(expT1[:], ps1[:], AF.Exp)
                # causal mask on GpSimd (DVE is the busier engine)
                nc.gpsimd.tensor_tensor(expT0[:], expT0[:], m0_s[:], OP.mult)
                nc.gpsimd.tensor_tensor(expT1[:], expT1[:], m1_s[:], OP.mult)

                # denominators, broadcast across partitions in interleaved
                # head form: rows 0-63 = even head, 64-127 = odd head
                rsb = hpool.tile([128, 3, T], f32, tag="rsb")
                for j in range(3):
                    dn = mmpool.tile([128, T], f32, tag="mm")
                    nc.tensor.matmul(
                        dn[:], cmL[:], expT0[:, 2 * j, :],
                        start=True, stop=False,
                    )
                    nc.tensor.matmul(
                        dn[:], cmR[:], expT0[:, 2 * j + 1, :],
                        start=False, stop=False,
                    )
                    nc.tensor.matmul(
                        dn[:, 128:200], cmL[0:72, :], expT1[:, 2 * j, :],
                        start=False, stop=False,
                    )
                    nc.tensor.matmul(
                        dn[:, 128:200], cmR[0:72, :], expT1[:, 2 * j + 1, :],
                        start=False, stop=True,
                    )
                    nc.vector.reciprocal_approx_fast(rsb[:, j, :], dn[:])

                # attV (unnormalized) into pa banks
                pa = papool.tile([128, 3, 256], f32, tag="pa")
                for j in range(3):
                    for r in range(2):
                        h = 2 * j + r
                        nc.tensor.matmul(
                            pa[64 * r : 64 * r + 64, j, 0:T],
                            v_all[0:128, b, 0, 64 * h : 64 * h + 64],
                            expT0[:, h, :],
                            start=True, stop=False,
                        )
                        nc.tensor.matmul(
                            pa[64 * r : 64 * r + 64, j, 128:200],
                            v_all[0:72, b, 1, 64 * h : 64 * h + 64],
                            expT1[:, h, :],
                            start=False, stop=True,
                        )
                # fused normalize: one multiply per batch
                nc.vector.tensor_tensor(
                    attT[:, :, c0 : c0 + T], pa[:, :, 0:T], rsb[:], OP.mult
                )

            # ---- proj + residual (in place into x_oct) ----
            for i in range(NT):
                w = TW[i]
                pp = mmpool.tile([128, E], f32, tag="mm")
                for k in range(3):
                    nc.tensor.matmul(
                        pp[:w, :],
                        attT[:, k, 128 * i : 128 * i + w],
                        wp_s[:, k, :],
                        start=(k == 0), stop=False,
                    )
                nc.tensor.matmul(
                    pp[:w, :], or_s[0:1, 0:w], bp_s[:],
                    start=False, stop=True,
                )
                nc.vector.tensor_tensor(
                    x_oct[:w, i, :], x_oct[:w, i, :], pp[:w, :], OP.add
                )

            # ---- LN2 + transpose ----
            h2 = spool.tile([128, NT, E], bf16, tag="h")
            layernorm(x_oct, h2)
            h2T = spool.tile([128, 3, TOK], bf16, tag="hT")
            transpose_feat(h2, h2T)

            # ---- FFN1 + ReLU (split ACT/DVE by chunk parity) ----
            uT = spool.tile([128, 12, TOK], bf16, tag="uT")
            for m in range(12):
                for c in range(NCH):
                    pu = mmpool.tile([128, CH], f32, tag="mm")
                    for k in range(3):
                        nc.tensor.matmul(
                            pu[:],
                            w1_s[:, k, 128 * m : 128 * (m + 1)],
                            h2T[:, k, CH * c : CH * (c + 1)],
                            start=(k == 0), stop=(k == 2),
                        )
                    if c % 2 == 0:
                        nc.scalar.activation(
                            uT[:, m, CH * c : CH * (c + 1)], pu[:],
                            AF.Relu, bias=b1_s[:, m : m + 1],
                        )
                    else:
                        nc.vector.tensor_scalar(
                            uT[:, m, CH * c : CH * (c + 1)], pu[:],
                            b1_s[:, m : m + 1], 0.0, OP.add, OP.max,
                        )

            # ---- FFN2 + residual (in place) + store ----
            for i in range(NT):
                w = TW[i]
                pf = mmpool.tile([128, E], f32, tag="mm")
                for k in range(12):
                    nc.tensor.matmul(
                        pf[:w, :],
                        uT[:, k, 128 * i : 128 * i + w],
                        w2_s[:, k, :],
                        start=(k == 0), stop=False,
                    )
                nc.tensor.matmul(
                    pf[:w, :], or_s[0:1, 0:w], b2_s[:],
                    start=False, stop=True,
                )
                nc.vector.tensor_tensor(
                    x_oct[:w, i, :], x_oct[:w, i, :], pf[:w, :], OP.add
                )
            nc.sync.dma_start(
                y_flat[r0 : r0 + 1536].rearrange("(g p) d -> p g d", p=128),
                x_oct[:, 0:12, :],
            )
            nc.sync.dma_start(y_flat[r0 + 1536 : r0 + 1600], x_oct[0:64, 12, :])

        if loop_cm is not None:
            loop_cm.__exit__(None, None, None)

    return nc


def _prep_inputs(inputs):
    """Host-side folding of LN gains/biases into weights. Exact in fp32."""
    bf = ml_dtypes.bfloat16
    x = np.asarray(inputs["x"], np.float32)
    Wq = np.asarray(inputs["Wq"], np.float32)
    Wk = np.asarray(inputs["Wk"], np.float32)
    Wv = np.asarray(inputs["Wv"], np.float32)
    Wp = np.asarray(inputs["Wproj"], np.float32)
    bproj = np.asarray(inputs["bproj"], np.float32)
    W1 = np.asarray(inputs["W1"], np.float32)
    b1 = np.asarray(inputs["b1"], np.float32)
    W2 = np.asarray(inputs["W2"], np.float32)
    b2 = np.asarray(inputs["b2"], np.float32)
    g1 = np.asarray(inputs["g1"], np.float32)
    be1 = np.asarray(inputs["be1"], np.float32)
    g2 = np.asarray(inputs["g2"], np.float32)
    be2 = np.asarray(inputs["be2"], np.float32)

    s = E ** -0.5
    wq_f = (g1[:, None] * Wq) * s
    wk_f = g1[:, None] * Wk
    wv_f = g1[:, None] * Wv
    cq = (be1 @ Wq) * s
    ck = be1 @ Wk
    cv = be1 @ Wv
    bp_f = bproj + cv @ Wp
    w1_f = g2[:, None] * W1
    b1_f = b1 + be2 @ W1

    m0 = np.zeros((128, NH, T), np.float32)
    sidx = np.arange(128)[:, None]
    tidx = np.arange(T)[None, :]
    m0[:, :, :] = (tidx >= sidx)[:, None, :]
    m1 = np.zeros((72, NH, 72), np.float32)
    si = np.arange(72)[:, None]
    ti = np.arange(72)[None, :]
    m1[:, :, :] = (ti >= si)[:, None, :]

    common = {
        "wq": wq_f.astype(bf), "wk": wk_f.astype(bf), "wv": wv_f.astype(bf),
        "wp": Wp.astype(bf), "w1": w1_f.astype(bf), "w2": W2.astype(bf),
        "cq": cq, "ck": ck, "b1p": b1_f,
        "bpb": bp_f.astype(bf).reshape(1, E), "b2b": b2.astype(bf).reshape(1, E),
        "m0": m0.astype(bf), "m1": m1.astype(bf),
        "onr": np.ones((1, 128), bf),
    }
    return x, common


def kernel(**inputs):
    from concourse.bass_utils import run_bass_kernel_spmd

    _install_wait_split_patch()

    x, common = _prep_inputs(inputs)
    if "nc" not in _CACHE:
        _CACHE["nc"] = _build_nc()
    nc = _CACHE["nc"]
    in_maps = []
    for c in range(NCORES):
        m = dict(common)
        m["x"] = np.ascontiguousarray(x[c * BPC : (c + 1) * BPC])
        in_maps.append(m)
    res = run_bass_kernel_spmd(nc, in_maps, core_ids=list(range(NCORES)))
    out = np.concatenate([res.results[c]["y"] for c in range(NCORES)], axis=0)
    return out.astype(np.float32)
